# revision 11
# baseline (speedup 1.0000x reference)
"""Trainium2 Bass kernel for an 8-head MultiHeadAttention with softmax over
the HEAD axis (not the key axis), returning (out, attention).

Sharding: data-parallel over batch. 16 batches / 8 cores = 2 batches per core.
No collectives needed (the head-axis softmax is local to each (b, q, k)).

Per-core pipeline (per batch):
  A) x -> x^T (PE transpose); projections Q^T_h, K^T_h [96, S] per head,
     V [S, 768] (bf16) via f32r matmuls; biases fused into PSUM evacuation.
  B) [q, k] layout: S_h = Q_h K_h^T (PE) -> E = exp(scale*S) (ACT) ->
     T = sum_h E (DVE strided reduce) -> R = 1/T (DVE) ->
     A = E * R (DVE fused) -> DMA to attention output.
  C) [k, q] layout: R^T via PE transpose of R; S^T_h (PE) ->
     A^T = exp(scale*S^T) * R^T (ACT+DVE, bf16) -> context_h += V_h^T A^T
     accumulated in PSUM over k -> out = context @ Wo + bo (per-head Wo tiles).
"""

import math
from contextlib import ExitStack

import numpy as np

import concourse.bass as bass
import concourse.mybir as mybir
import concourse.tile as tile
from concourse import bacc
from concourse.bass import ts
from concourse.bass_utils import run_bass_kernel_spmd
from concourse.masks import make_identity

F32 = mybir.dt.float32
F32R = mybir.dt.float32r
BF16 = mybir.dt.bfloat16
FP = F32

EMB = 768
HEADS = 8
HD = 96  # head dim
N_CORES = 8

AF = mybir.ActivationFunctionType
ALU = mybir.AluOpType


def _mm(nc, out, lhsT, rhs, start, stop):
    """f32r matmul: both operands are fp32 tiles bitcast to float32r."""
    nc.tensor.matmul(
        out,
        lhsT.bitcast(F32R),
        rhs.bitcast(F32R),
        start=start,
        stop=stop,
    )


def build_kernel(nc, seq=1024, bpc=2, av_bf16=True):
    """Emit the whole SPMD per-core program into `nc` (a Bacc instance)."""
    S = seq
    NQT = S // 128   # q tiles of 128
    NKH = S // 512   # k halves of 512
    NCH = EMB // 128  # contraction chunks (6)
    scale = 1.0 / math.sqrt(HD)
    av_dt = BF16 if av_bf16 else F32R

    x_d = nc.dram_tensor("x", [bpc, S, EMB], FP, kind="ExternalInput").ap()
    w_d = {}
    b_d = {}
    for nm in ("q", "k", "v", "o"):
        w_d[nm] = nc.dram_tensor(f"W{nm}", [EMB, EMB], FP, kind="ExternalInput").ap()
        b_d[nm] = nc.dram_tensor(f"b{nm}", [EMB], FP, kind="ExternalInput").ap()
    out_d = nc.dram_tensor("out", [bpc, S, EMB], FP, kind="ExternalOutput").ap()
    attn_d = nc.dram_tensor(
        "attn", [bpc, HEADS, S, S], FP, kind="ExternalOutput"
    ).ap()

    with tile.TileContext(nc) as tc, ExitStack() as ctx:
        # ---------- persistent constants ----------
        const_pool = ctx.enter_context(tc.tile_pool(name="const", bufs=1))
        ident = const_pool.tile([128, 128], FP, name="ident")
        make_identity(nc, ident[:])
        ones_col = const_pool.tile([1, 128], FP, name="ones_col")
        nc.gpsimd.memset(ones_col[:], 1.0)

        # bias row tiles [1, 768]
        brow = {}
        for nm in ("q", "k", "v", "o"):
            t = const_pool.tile([1, EMB], FP, name=f"brow_{nm}")
            nc.sync.dma_start(out=t[:], in_=b_d[nm].unsqueeze(0))
            brow[nm] = t
        # bq/bk as [96, 8] per-partition-scalar tiles (column h = bias of head h)
        bcolT = {}
        for nm in ("q", "k"):
            t = const_pool.tile([HD, HEADS], FP, name=f"bcolT_{nm}")
            nc.sync.dma_start(out=t[:], in_=b_d[nm].rearrange("(h d) -> d h", d=HD))
            bcolT[nm] = t
        # replicated bias tiles [128, 768] for free-dim bias adds (V, out)
        brep = {}
        with tc.tile_pool(name="brep_psum", bufs=2, space="PSUM") as bp_pool:
            for nm in ("v", "o"):
                rep = const_pool.tile([128, EMB], FP, name=f"brep_{nm}")
                for half in range(2):
                    ps = bp_pool.tile([128, 384], FP, name="brep_ps", tag="brep_ps")
                    nc.tensor.matmul(
                        ps[:],
                        ones_col[:],
                        brow[nm][:, ts(half, 384)],
                        start=True,
                        stop=True,
                    )
                    nc.vector.tensor_copy(rep[:, ts(half, 384)], ps[:])
                brep[nm] = rep

        # per-head Wo tiles [96, 768]
        wo_h = []
        for h in range(HEADS):
            t = const_pool.tile([HD, EMB], F32R, name=f"wo_{h}")
            nc.sync.dma_start(out=t[:], in_=w_d["o"][ts(h, HD), :].bitcast(F32R))
            wo_h.append(t)

        for b in range(bpc):
            _build_batch(
                tc, nc, b, S, NQT, NKH, NCH, scale, av_dt,
                x_d, w_d, out_d, attn_d,
                ident, brow, bcolT, brep, wo_h,
            )


def _build_batch(
    tc, nc, b, S, NQT, NKH, NCH, scale, av_dt,
    x_d, w_d, out_d, attn_d,
    ident, brow, bcolT, brep, wo_h,
):
    with ExitStack() as bctx:
        # ---------- persistent per-batch tensors ----------
        perm = bctx.enter_context(tc.tile_pool(name=f"perm{b}", bufs=1))
        qT = [perm.tile([HD, S], F32R, name=f"qT{b}_{h}") for h in range(HEADS)]
        kT = [perm.tile([HD, S], F32R, name=f"kT{b}_{h}") for h in range(HEADS)]
        v_t = [perm.tile([128, EMB], av_dt, name=f"v{b}_{i}") for i in range(NQT)]
        rbuf = [perm.tile([128, S], FP, name=f"r{b}_{i}") for i in range(NQT)]

        # ================= stage A: x^T + projections =================
        with ExitStack() as actx:
            xT_pool = actx.enter_context(tc.tile_pool(name=f"xT{b}", bufs=1))
            aps = actx.enter_context(
                tc.tile_pool(name=f"sa_ps{b}", bufs=2, space="PSUM")
            )
            xT = [xT_pool.tile([128, S], F32R, name=f"xT{b}_{c}") for c in range(NCH)]

            # load x tiles and transpose into xT
            with tc.tile_pool(name=f"xt{b}", bufs=3) as xtpool:
                for i in range(NQT):
                    xt = xtpool.tile([128, EMB], FP, name="xt", tag="xt", bufs=3)
                    nc.sync.dma_start(out=xt[:], in_=x_d[b, ts(i, 128), :])
                    for c in range(NCH):
                        ps = aps.tile([128, 128], FP, name="tp_ps", tag="a_ps", bufs=6)
                        nc.tensor.transpose(ps[:], xt[:, ts(c, 128)], ident[:])
                        nc.scalar.copy(xT[c][:, ts(i, 128)], ps[:])

            # Q^T, K^T projections (per head, M=96)
            for nm, dst in (("q", qT), ("k", kT)):
                with tc.tile_pool(name=f"w{nm}{b}", bufs=1) as wpool:
                    w_t = [
                        wpool.tile([128, EMB], F32R, name=f"w{nm}_{c}")
                        for c in range(NCH)
                    ]
                    for c in range(NCH):
                        nc.sync.dma_start(
                            out=w_t[c][:], in_=w_d[nm][ts(c, 128), :].bitcast(F32R)
                        )
                    for h in range(HEADS):
                        for half in range(S // 512):
                            ps = aps.tile(
                                [HD, 512], FP, name="proj_ps", tag="a_ps", bufs=6
                            )
                            for c in range(NCH):
                                _mm(
                                    nc, ps[:],
                                    w_t[c][:, ts(h, HD)],
                                    xT[c][:, ts(half, 512)],
                                    start=(c == 0), stop=(c == NCH - 1),
                                )
                            nc.vector.tensor_scalar(
                                out=dst[h][:, ts(half, 512)],
                                in0=ps[:],
                                scalar1=bcolT[nm][:, h : h + 1],
                                scalar2=None,
                                op0=ALU.add,
                            )

            # V projection (natural layout, bf16, M=t-tile)
            with tc.tile_pool(name=f"wv{b}", bufs=1) as wpool:
                w_t = [
                    wpool.tile([128, EMB], F32R, name=f"wv_{c}") for c in range(NCH)
                ]
                for c in range(NCH):
                    nc.sync.dma_start(
                        out=w_t[c][:], in_=w_d["v"][ts(c, 128), :].bitcast(F32R)
                    )
                for i in range(NQT):
                    for half in range(2):
                        ps = aps.tile(
                            [128, 384], FP, name="v_ps", tag="a_ps", bufs=6
                        )
                        for c in range(NCH):
                            _mm(
                                nc, ps[:],
                                xT[c][:, ts(i, 128)],
                                w_t[c][:, ts(half, 384)],
                                start=(c == 0), stop=(c == NCH - 1),
                            )
                        nc.vector.scalar_tensor_tensor(
                            out=v_t[i][:, ts(half, 384)],
                            in0=ps[:],
                            scalar=1.0,
                            in1=brep["v"][:, ts(half, 384)],
                            op0=ALU.mult,
                            op1=ALU.add,
                        )

        # ================= stage B: A = softmax_h, [q,k] layout =================
        with ExitStack() as bctx2:
            bpool = bctx2.enter_context(tc.tile_pool(name=f"sb{b}", bufs=2))
            bps = bctx2.enter_context(
                tc.tile_pool(name=f"sb_ps{b}", bufs=2, space="PSUM")
            )
            for qt in range(NQT):
                for kh in range(NKH):
                    e_all = bpool.tile(
                        [128, HEADS * 512], FP, name="e_all", tag="e_all", bufs=2
                    )
                    for hg in range(2):  # 4 heads per PSUM group
                        ps = bps.tile(
                            [128, 4 * 512], FP, name="s_ps", tag="s_ps", bufs=2
                        )
                        for hh in range(4):
                            h = hg * 4 + hh
                            _mm(
                                nc, ps[:, ts(hh, 512)],
                                qT[h][:, ts(qt, 128)],
                                kT[h][:, ts(kh, 512)],
                                start=True, stop=True,
                            )
                        nc.scalar.activation(
                            e_all[:, ts(hg, 4 * 512)], ps[:], AF.Exp, scale=scale
                        )
                    # head-sum: view (h k) -> [k, h], reduce innermost h
                    t_sum = bpool.tile([128, 512], FP, name="t_sum", tag="t_sum", bufs=2)
                    nc.vector.reduce_sum(
                        out=t_sum[:],
                        in_=e_all[:].rearrange("p (h k) -> p k h", h=HEADS),
                        axis=mybir.AxisListType.X,
                    )
                    r_slice = rbuf[qt][:, ts(kh, 512)]
                    nc.vector.reciprocal(r_slice, t_sum[:])
                    nc.vector.scalar_tensor_tensor(
                        out=e_all[:],
                        in0=e_all[:],
                        scalar=1.0,
                        in1=r_slice.unsqueeze(1).broadcast_to([128, HEADS, 512]),
                        op0=ALU.mult,
                        op1=ALU.mult,
                    )
                    nc.sync.dma_start(
                        out=attn_d[b, :, ts(qt, 128), ts(kh, 512)].rearrange(
                            "h q k -> q h k"
                        ),
                        in_=e_all[:].rearrange("p (h k) -> p h k", h=HEADS),
                    )

        # ================= R^T phase =================
        with ExitStack() as rctx:
            rT = rbuf  # transposed in place, block-pairwise
            with tc.tile_pool(name=f"rT_ps{b}", bufs=4, space="PSUM") as rps:
                for i in range(NQT):
                    for j in range(i, NQT):
                        ps = rps.tile([128, 256], FP, name="rt_ps", tag="rt_ps", bufs=4)
                        nc.tensor.transpose(
                            ps[:, ts(0, 128)], rbuf[i][:, ts(j, 128)], ident[:]
                        )
                        if j > i:
                            nc.tensor.transpose(
                                ps[:, ts(1, 128)], rbuf[j][:, ts(i, 128)], ident[:]
                            )
                        if j > i:
                            nc.scalar.copy(rbuf[i][:, ts(j, 128)], ps[:, ts(1, 128)])
                            nc.vector.tensor_copy(
                                rbuf[j][:, ts(i, 128)], ps[:, ts(0, 128)]
                            )
                        else:
                            nc.scalar.copy(rbuf[i][:, ts(j, 128)], ps[:, ts(0, 128)])

            # ================= stage C: A^T, AV, out =================
            with ExitStack() as cctx:
                cpool = cctx.enter_context(tc.tile_pool(name=f"sc{b}", bufs=2))
                sps = cctx.enter_context(
                    tc.tile_pool(name=f"sc_ps{b}", bufs=2, space="PSUM")
                )
                cxps = cctx.enter_context(
                    tc.tile_pool(name=f"cx_ps{b}", bufs=2, space="PSUM")
                )
                for qh in range(S // 512):
                    ctxT = cpool.tile(
                        [HD, HEADS * 512], F32R, name="ctxT", tag="ctxT", bufs=1
                    )
                    for hg in range(HEADS // 2):  # 2 heads per group
                        cps = [
                            cxps.tile([HD, 512], FP, name="c_ps", tag=f"c_ps{j}")
                            for j in range(2)
                        ]
                        for kc in range(S // 128):
                            ps = sps.tile(
                                [128, 2 * 512], FP, name="st_ps", tag="st_ps", bufs=2
                            )
                            for j in range(2):
                                h = hg * 2 + j
                                _mm(
                                    nc, ps[:, ts(j, 512)],
                                    kT[h][:, ts(kc, 128)],
                                    qT[h][:, ts(qh, 512)],
                                    start=True, stop=True,
                                )
                            eT = cpool.tile(
                                [128, 2 * 512], av_dt, name="eT", tag="eT", bufs=2
                            )
                            nc.scalar.activation(eT[:], ps[:], AF.Exp, scale=scale)
                            aT = cpool.tile(
                                [128, 2 * 512], av_dt, name="aT", tag="aT", bufs=2
                            )
                            nc.vector.scalar_tensor_tensor(
                                out=aT[:],
                                in0=eT[:],
                                scalar=1.0,
                                in1=rT[kc][:, ts(qh, 512)]
                                .unsqueeze(1)
                                .broadcast_to([128, 2, 512]),
                                op0=ALU.mult,
                                op1=ALU.mult,
                            )
                            for j in range(2):
                                h = hg * 2 + j
                                va = v_t[kc][:, ts(h, HD)]
                                ata = aT[:, ts(j, 512)]
                                nc.tensor.matmul(
                                    cps[j][:],
                                    va,
                                    ata,
                                    start=(kc == 0),
                                    stop=(kc == S // 128 - 1),
                                )
                        for j in range(2):
                            h = hg * 2 + j
                            nc.scalar.copy(ctxT[:, ts(h, 512)], cps[j][:])

                    # out projection for this q-half
                    for i4 in range(4):
                        i = qh * 4 + i4
                        o_sb = cpool.tile(
                            [128, EMB], FP, name="o_sb", tag="o_sb", bufs=3
                        )
                        for half in range(2):
                            ps = sps.tile(
                                [128, 384], FP, name="o_ps", tag="st_ps", bufs=2
                            )
                            for h in range(HEADS):
                                _mm(
                                    nc, ps[:],
                                    ctxT[:, ts(h, 512)][:, ts(i4, 128)],
                                    wo_h[h][:, ts(half, 384)],
                                    start=(h == 0), stop=(h == HEADS - 1),
                                )
                            nc.vector.scalar_tensor_tensor(
                                out=o_sb[:, ts(half, 384)],
                                in0=ps[:],
                                scalar=1.0,
                                in1=brep["o"][:, ts(half, 384)],
                                op0=ALU.mult,
                                op1=ALU.add,
                            )
                        nc.sync.dma_start(
                            out=out_d[b, ts(i, 128), :], in_=o_sb[:]
                        )


_CACHE = {}
TRACE = False          # set by test harnesses to capture an NTFF profile
LAST_RESULT = None     # BassKernelResults of the most recent kernel() call


def _get_nc(seq=1024, bpc=2):
    key = (seq, bpc)
    if key not in _CACHE:
        nc = bacc.Bacc(
            "TRN2", target_bir_lowering=False, debug=False, num_devices=N_CORES
        )
        build_kernel(nc, seq=seq, bpc=bpc)
        nc.compile()
        _CACHE[key] = nc
    return _CACHE[key]


def kernel(x, Wq, bq, Wk, bk, Wv, bv, Wo, bo):
    x = np.asarray(x, dtype=np.float32)
    B, S, E = x.shape
    bpc = B // N_CORES
    nc = _get_nc(seq=S, bpc=bpc)

    common = {
        "Wq": np.asarray(Wq, np.float32), "bq": np.asarray(bq, np.float32),
        "Wk": np.asarray(Wk, np.float32), "bk": np.asarray(bk, np.float32),
        "Wv": np.asarray(Wv, np.float32), "bv": np.asarray(bv, np.float32),
        "Wo": np.asarray(Wo, np.float32), "bo": np.asarray(bo, np.float32),
    }
    in_maps = [
        {"x": x[c * bpc : (c + 1) * bpc], **common} for c in range(N_CORES)
    ]
    global LAST_RESULT
    res = run_bass_kernel_spmd(nc, in_maps, list(range(N_CORES)), trace=TRACE)
    LAST_RESULT = res
    out = np.concatenate([r["out"] for r in res.results], axis=0)
    attn = np.concatenate([r["attn"] for r in res.results], axis=0)
    return out, attn


# revision 13
# speedup vs baseline: 1.1284x; 1.1284x over previous
"""Trainium2 Bass kernel for an 8-head MultiHeadAttention with softmax over
the HEAD axis (not the key axis), returning (out, attention).

Sharding: data-parallel over batch. 16 batches / 8 cores = 2 batches per core.
No collectives needed (the head-axis softmax is local to each (b, q, k)).

Per-core pipeline (per batch):
  A) x -> x^T (PE transpose); projections Q^T_h, K^T_h [96, S] per head,
     V [S, 768] (bf16) via f32r matmuls; biases fused into PSUM evacuation.
  B) [q, k] layout: S_h = Q_h K_h^T (PE) -> E = exp(scale*S) (ACT) ->
     T = sum_h E (tree adds, GPSIMD leaves + DVE upper) ->
     R = 1/T (DVE reciprocal_approx_fast) ->
     A = E * R in place (DVE fused) -> DMA to attention output.
  R) R^T via pairwise PE block transposes -> bf16 rT tiles.
  C) [k, q] layout: S^T_h (PE) -> A^T = exp(scale*S^T) * R^T (ACT + DVE
     all-bf16 2x) -> context_h += V_h^T A^T accumulated in PSUM over k ->
     out = context @ Wo + bo (per-head Wo tiles, contraction chained over h).
"""

import math
from contextlib import ExitStack

import numpy as np

import concourse.bass as bass
import concourse.mybir as mybir
import concourse.tile as tile
from concourse import bacc
from concourse.bass import ts
from concourse.bass_utils import run_bass_kernel_spmd
from concourse.masks import make_identity

F32 = mybir.dt.float32
F32R = mybir.dt.float32r
BF16 = mybir.dt.bfloat16
FP = F32

EMB = 768
HEADS = 8
HD = 96  # head dim
N_CORES = 8

AF = mybir.ActivationFunctionType
ALU = mybir.AluOpType


def _mm(nc, out, lhsT, rhs, start, stop):
    nc.tensor.matmul(out, lhsT, rhs, start=start, stop=stop)


def build_kernel(nc, seq=1024, bpc=2, av_bf16=True):
    """Emit the whole SPMD per-core program into `nc` (a Bacc instance)."""
    S = seq
    NQT = S // 128   # q/k tiles of 128
    NKH = S // 512   # k halves of 512
    NCH = EMB // 128  # contraction chunks (6)
    scale = 1.0 / math.sqrt(HD)
    av_dt = BF16 if av_bf16 else F32R

    x_d = nc.dram_tensor("x", [bpc, S, EMB], FP, kind="ExternalInput").ap()
    w_d = {}
    b_d = {}
    for nm in ("q", "k", "v", "o"):
        w_d[nm] = nc.dram_tensor(f"W{nm}", [EMB, EMB], FP, kind="ExternalInput").ap()
        b_d[nm] = nc.dram_tensor(f"b{nm}", [EMB], FP, kind="ExternalInput").ap()
    out_d = nc.dram_tensor("out", [bpc, S, EMB], FP, kind="ExternalOutput").ap()
    attn_d = nc.dram_tensor(
        "attn", [bpc, HEADS, S, S], FP, kind="ExternalOutput"
    ).ap()

    with tile.TileContext(nc) as tc, ExitStack() as ctx:
        # ---------- persistent constants ----------
        const_pool = ctx.enter_context(tc.tile_pool(name="const", bufs=1))
        ident = const_pool.tile([128, 128], FP, name="ident")
        make_identity(nc, ident[:])
        ones_col = const_pool.tile([1, 128], FP, name="ones_col")
        nc.gpsimd.memset(ones_col[:], 1.0)

        # bq/bk as [96, 8] per-partition-scalar tiles (column h = bias of head h)
        bcolT = {}
        for nm in ("q", "k"):
            t = const_pool.tile([HD, HEADS], FP, name=f"bcolT_{nm}")
            nc.sync.dma_start(out=t[:], in_=b_d[nm].rearrange("(h d) -> d h", d=HD))
            bcolT[nm] = t

        # replicated bias tiles [128, 768] for free-dim bias adds (V, out)
        brep = {}
        with (
            tc.tile_pool(name="brow_tmp", bufs=1) as brow_pool,
            tc.tile_pool(name="brep_psum", bufs=2, space="PSUM") as bp_pool,
        ):
            for nm in ("v", "o"):
                row = brow_pool.tile([1, EMB], FP, name=f"brow_{nm}", tag=f"br{nm}")
                nc.sync.dma_start(out=row[:], in_=b_d[nm].unsqueeze(0))
                rep = const_pool.tile([128, EMB], FP, name=f"brep_{nm}")
                for half in range(2):
                    ps = bp_pool.tile([128, 384], FP, name="brep_ps", tag="brep_ps")
                    nc.tensor.matmul(
                        ps[:], ones_col[:], row[:, ts(half, 384)],
                        start=True, stop=True,
                    )
                    nc.vector.tensor_copy(rep[:, ts(half, 384)], ps[:])
                brep[nm] = rep

        for b in range(bpc):
            _build_batch(
                tc, nc, b, S, NQT, NKH, NCH, scale, av_dt,
                x_d, w_d, b_d, out_d, attn_d,
                ident, bcolT, brep,
            )


def _build_batch(
    tc, nc, b, S, NQT, NKH, NCH, scale, av_dt,
    x_d, w_d, b_d, out_d, attn_d,
    ident, bcolT, brep,
):
    with ExitStack() as bctx:
        # ---------- persistent per-batch tensors ----------
        perm = bctx.enter_context(tc.tile_pool(name=f"perm{b}", bufs=1))
        qT = [perm.tile([HD, S], F32R, name=f"qT{b}_{h}") for h in range(HEADS)]
        kT = [perm.tile([HD, S], F32R, name=f"kT{b}_{h}") for h in range(HEADS)]
        v_t = [perm.tile([128, EMB], av_dt, name=f"v{b}_{i}") for i in range(NQT)]

        # ================= stage A: x^T + projections =================
        with ExitStack() as actx:
            xT_pool = actx.enter_context(tc.tile_pool(name=f"xT{b}", bufs=1))
            aps = actx.enter_context(
                tc.tile_pool(name=f"sa_ps{b}", bufs=2, space="PSUM")
            )
            xT = [xT_pool.tile([128, S], F32R, name=f"xT{b}_{c}") for c in range(NCH)]

            # load x tiles and transpose into xT
            with tc.tile_pool(name=f"xt{b}", bufs=3) as xtpool:
                for i in range(NQT):
                    xt = xtpool.tile([128, EMB], FP, name="xt", tag="xt", bufs=3)
                    nc.sync.dma_start(out=xt[:], in_=x_d[b, ts(i, 128), :])
                    for c in range(NCH):
                        ps = aps.tile([128, 128], FP, name="tp_ps", tag="a_ps", bufs=6)
                        nc.tensor.transpose(ps[:], xt[:, ts(c, 128)], ident[:])
                        if (i + c) % 2:
                            nc.scalar.copy(xT[c][:, ts(i, 128)], ps[:])
                        else:
                            nc.vector.tensor_copy(xT[c][:, ts(i, 128)], ps[:])

            # Q^T, K^T projections (per head, M=96)
            for nm, dst in (("q", qT), ("k", kT)):
                with tc.tile_pool(name=f"w{nm}{b}", bufs=1) as wpool:
                    w_t = [
                        wpool.tile([128, EMB], F32R, name=f"w{nm}_{c}")
                        for c in range(NCH)
                    ]
                    for c in range(NCH):
                        nc.sync.dma_start(
                            out=w_t[c][:], in_=w_d[nm][ts(c, 128), :].bitcast(F32R)
                        )
                    for h in range(HEADS):
                        for half in range(S // 512):
                            ps = aps.tile(
                                [HD, 512], FP, name="proj_ps", tag="a_ps", bufs=6
                            )
                            for c in range(NCH):
                                _mm(
                                    nc, ps[:],
                                    w_t[c][:, ts(h, HD)],
                                    xT[c][:, ts(half, 512)],
                                    start=(c == 0), stop=(c == NCH - 1),
                                )
                            nc.vector.tensor_scalar(
                                out=dst[h][:, ts(half, 512)],
                                in0=ps[:],
                                scalar1=bcolT[nm][:, h : h + 1],
                                scalar2=None,
                                op0=ALU.add,
                            )

            # V projection (natural layout, M=t-tile)
            with tc.tile_pool(name=f"wv{b}", bufs=1) as wpool:
                w_t = [
                    wpool.tile([128, EMB], F32R, name=f"wv_{c}") for c in range(NCH)
                ]
                for c in range(NCH):
                    nc.sync.dma_start(
                        out=w_t[c][:], in_=w_d["v"][ts(c, 128), :].bitcast(F32R)
                    )
                for i in range(NQT):
                    for half in range(2):
                        ps = aps.tile(
                            [128, 384], FP, name="v_ps", tag="a_ps", bufs=6
                        )
                        for c in range(NCH):
                            _mm(
                                nc, ps[:],
                                xT[c][:, ts(i, 128)],
                                w_t[c][:, ts(half, 384)],
                                start=(c == 0), stop=(c == NCH - 1),
                            )
                        nc.vector.scalar_tensor_tensor(
                            out=v_t[i][:, ts(half, 384)],
                            in0=ps[:],
                            scalar=1.0,
                            in1=brep["v"][:, ts(half, 384)],
                            op0=ALU.mult,
                            op1=ALU.add,
                        )

        # rT (bf16 transposed reciprocal) outlives rbuf; rbuf closes before C
        rT_pool = bctx.enter_context(tc.tile_pool(name=f"rT{b}", bufs=1))
        rT = [rT_pool.tile([128, S], BF16, name=f"rT{b}_{i}") for i in range(NQT)]

        with ExitStack() as rctx:
            rbuf_pool = rctx.enter_context(tc.tile_pool(name=f"rb{b}", bufs=1))
            rbuf = [
                rbuf_pool.tile([128, S], FP, name=f"r{b}_{i}") for i in range(NQT)
            ]

            # ======== stage B: A = softmax_h, [q,k] layout ========
            with ExitStack() as bctx2:
                bpool = bctx2.enter_context(tc.tile_pool(name=f"sb{b}", bufs=2))
                bps = bctx2.enter_context(
                    tc.tile_pool(name=f"sb_ps{b}", bufs=2, space="PSUM")
                )
                for qt in range(NQT):
                    for kh in range(NKH):
                        e_all = bpool.tile(
                            [128, HEADS * 512], FP, name="e_all",
                            tag="e_all", bufs=2,
                        )
                        for hg in range(2):  # 4 heads per PSUM group
                            ps = bps.tile(
                                [128, 4 * 512], FP, name="s_ps",
                                tag="s_ps", bufs=2,
                            )
                            for hh in range(4):
                                h = hg * 4 + hh
                                _mm(
                                    nc, ps[:, ts(hh, 512)],
                                    qT[h][:, ts(qt, 128)],
                                    kT[h][:, ts(kh, 512)],
                                    start=True, stop=True,
                                )
                            nc.scalar.activation(
                                e_all[:, ts(hg, 4 * 512)], ps[:],
                                AF.Exp, scale=scale,
                            )
                        # head-sum tree: leaf adds on GpSimd, upper on DVE
                        ptmp = bpool.tile(
                            [128, 2 * 512], FP, name="ptmp", tag="ptmp", bufs=2
                        )
                        t_sum = bpool.tile(
                            [128, 512], FP, name="t_sum", tag="t_sum", bufs=2
                        )
                        eh = lambda h: e_all[:, ts(h, 512)]
                        nc.gpsimd.tensor_tensor(
                            ptmp[:, ts(0, 512)], eh(0), eh(1), ALU.add
                        )
                        nc.gpsimd.tensor_tensor(
                            ptmp[:, ts(1, 512)], eh(2), eh(3), ALU.add
                        )
                        nc.vector.tensor_tensor(t_sum[:], eh(4), eh(5), ALU.add)
                        nc.gpsimd.tensor_tensor(
                            ptmp[:, ts(0, 512)], ptmp[:, ts(0, 512)],
                            ptmp[:, ts(1, 512)], ALU.add,
                        )
                        nc.vector.tensor_tensor(t_sum[:], t_sum[:], eh(6), ALU.add)
                        nc.vector.tensor_tensor(t_sum[:], t_sum[:], eh(7), ALU.add)
                        nc.vector.tensor_tensor(
                            t_sum[:], t_sum[:], ptmp[:, ts(0, 512)], ALU.add
                        )
                        r_slice = rbuf[qt][:, ts(kh, 512)]
                        nc.vector.reciprocal_approx_fast(out=r_slice, in_=t_sum[:])
                        nc.vector.scalar_tensor_tensor(
                            out=e_all[:],
                            in0=e_all[:],
                            scalar=1.0,
                            in1=r_slice.unsqueeze(1).broadcast_to([128, HEADS, 512]),
                            op0=ALU.mult,
                            op1=ALU.mult,
                        )
                        nc.sync.dma_start(
                            out=attn_d[b, :, ts(qt, 128), ts(kh, 512)].rearrange(
                                "h q k -> q h k"
                            ),
                            in_=e_all[:].rearrange("p (h k) -> p h k", h=HEADS),
                        )

            # ======== R^T phase: rbuf (fp32) -> rT (bf16) ========
            with tc.tile_pool(name=f"rT_ps{b}", bufs=4, space="PSUM") as rps:
                for i in range(NQT):
                    for j4 in range(NQT // 4):
                        ps = rps.tile(
                            [128, 512], FP, name="rt_ps", tag="rt_ps", bufs=4
                        )
                        for j in range(4):
                            jj = j4 * 4 + j
                            nc.tensor.transpose(
                                ps[:, ts(j, 128)],
                                rbuf[jj][:, ts(i, 128)],
                                ident[:],
                            )
                        if (i + j4) % 2:
                            nc.scalar.copy(rT[i][:, ts(j4, 512)], ps[:])
                        else:
                            nc.vector.tensor_copy(rT[i][:, ts(j4, 512)], ps[:])

        # ================= stage C: A^T, AV, out =================
        with ExitStack() as cctx:
            wo_pool = cctx.enter_context(tc.tile_pool(name=f"wo{b}", bufs=1))
            wo_h = []
            for h in range(HEADS):
                t = wo_pool.tile([HD, EMB], F32R, name=f"wo{b}_{h}")
                nc.sync.dma_start(
                    out=t[:], in_=w_d["o"][ts(h, HD), :].bitcast(F32R)
                )
                wo_h.append(t)
            cpool = cctx.enter_context(tc.tile_pool(name=f"sc{b}", bufs=2))
            sps = cctx.enter_context(
                tc.tile_pool(name=f"sc_ps{b}", bufs=2, space="PSUM")
            )
            cxps = cctx.enter_context(
                tc.tile_pool(name=f"cx_ps{b}", bufs=2, space="PSUM")
            )
            for qh in range(S // 512):
                ctxT = cpool.tile(
                    [HD, HEADS * 512], F32R, name="ctxT", tag="ctxT", bufs=1
                )
                for hg in range(HEADS // 2):  # 2 heads per group
                    cps = [
                        cxps.tile([HD, 512], FP, name="c_ps", tag=f"c_ps{j}")
                        for j in range(2)
                    ]
                    for kc in range(S // 128):
                        ps = sps.tile(
                            [128, 2 * 512], FP, name="st_ps", tag="st_ps", bufs=2
                        )
                        for j in range(2):
                            h = hg * 2 + j
                            _mm(
                                nc, ps[:, ts(j, 512)],
                                kT[h][:, ts(kc, 128)],
                                qT[h][:, ts(qh, 512)],
                                start=True, stop=True,
                            )
                        eT = cpool.tile(
                            [128, 2 * 512], av_dt, name="eT", tag="eT", bufs=2
                        )
                        nc.scalar.activation(eT[:], ps[:], AF.Exp, scale=scale)
                        aT = cpool.tile(
                            [128, 2 * 512], av_dt, name="aT", tag="aT", bufs=2
                        )
                        nc.vector.scalar_tensor_tensor(
                            out=aT[:],
                            in0=eT[:],
                            scalar=1.0,
                            in1=rT[kc][:, ts(qh, 512)]
                            .unsqueeze(1)
                            .broadcast_to([128, 2, 512]),
                            op0=ALU.mult,
                            op1=ALU.mult,
                        )
                        for j in range(2):
                            h = hg * 2 + j
                            nc.tensor.matmul(
                                cps[j][:],
                                v_t[kc][:, ts(h, HD)],
                                aT[:, ts(j, 512)],
                                start=(kc == 0),
                                stop=(kc == S // 128 - 1),
                            )
                    for j in range(2):
                        h = hg * 2 + j
                        if hg % 2:
                            nc.scalar.copy(ctxT[:, ts(h, 512)], cps[j][:])
                        else:
                            nc.vector.tensor_copy(ctxT[:, ts(h, 512)], cps[j][:])

                # out projection for this q-half
                for i4 in range(4):
                    i = qh * 4 + i4
                    o_sb = cpool.tile(
                        [128, EMB], FP, name="o_sb", tag="o_sb", bufs=2
                    )
                    for half in range(2):
                        ps = sps.tile(
                            [128, 384], FP, name="o_ps", tag="st_ps", bufs=2
                        )
                        for h in range(HEADS):
                            _mm(
                                nc, ps[:],
                                ctxT[:, ts(h, 512)][:, ts(i4, 128)],
                                wo_h[h][:, ts(half, 384)],
                                start=(h == 0), stop=(h == HEADS - 1),
                            )
                        nc.vector.scalar_tensor_tensor(
                            out=o_sb[:, ts(half, 384)],
                            in0=ps[:],
                            scalar=1.0,
                            in1=brep["o"][:, ts(half, 384)],
                            op0=ALU.mult,
                            op1=ALU.add,
                        )
                    nc.sync.dma_start(out=out_d[b, ts(i, 128), :], in_=o_sb[:])


_CACHE = {}
TRACE = False          # set by test harnesses to capture an NTFF profile
LAST_RESULT = None     # BassKernelResults of the most recent kernel() call


def _get_nc(seq=1024, bpc=2):
    key = (seq, bpc)
    if key not in _CACHE:
        nc = bacc.Bacc(
            "TRN2", target_bir_lowering=False, debug=False, num_devices=N_CORES
        )
        build_kernel(nc, seq=seq, bpc=bpc)
        nc.compile()
        _CACHE[key] = nc
    return _CACHE[key]


def kernel(x, Wq, bq, Wk, bk, Wv, bv, Wo, bo):
    x = np.asarray(x, dtype=np.float32)
    B, S, E = x.shape
    bpc = B // N_CORES
    nc = _get_nc(seq=S, bpc=bpc)

    common = {
        "Wq": np.asarray(Wq, np.float32), "bq": np.asarray(bq, np.float32),
        "Wk": np.asarray(Wk, np.float32), "bk": np.asarray(bk, np.float32),
        "Wv": np.asarray(Wv, np.float32), "bv": np.asarray(bv, np.float32),
        "Wo": np.asarray(Wo, np.float32), "bo": np.asarray(bo, np.float32),
    }
    in_maps = [
        {"x": x[c * bpc : (c + 1) * bpc], **common} for c in range(N_CORES)
    ]
    global LAST_RESULT
    res = run_bass_kernel_spmd(nc, in_maps, list(range(N_CORES)), trace=TRACE)
    LAST_RESULT = res
    out = np.concatenate([r["out"] for r in res.results], axis=0)
    attn = np.concatenate([r["attn"] for r in res.results], axis=0)
    return out, attn
